# revision 8
# baseline (speedup 1.0000x reference)
"""Self-contained 8-core Trainium2 Bass kernel for nn_MultiHeadAttention.

Sharding: core c = (b, g), b = c // 4 (batch), g = c % 4 (kv head group).
Each core computes heads 4g..4g+3 for batch b (they share kv head g).

Cross-core traffic stays on-device: x[b] arrives as per-core S-quarters
(AllGather over the 4-core batch group rebuilds the full [M, S] operand),
and the per-group partial outputs through the Wo row-slice are summed with
an on-device ReduceScatter, so each core emits a disjoint bf16 [S/4, M]
slice of the final output. This keeps host<->device transfer (the axon
tunnel, which dominates wall-clock) near the information-theoretic floor.
"""
import numpy as np
import ml_dtypes

import concourse.bass as bass
import concourse.mybir as mybir
import concourse.tile as tile
from concourse import bass_utils

F32 = mybir.dt.float32
BF16 = mybir.dt.bfloat16
ALU = mybir.AluOpType
ACT = mybir.ActivationFunctionType

B, S, M, H, HKV, D = 2, 2048, 1024, 16, 4, 64
HL = H // HKV          # local q heads per core = 4
SQ = S // 4            # per-core sequence quarter = 512
PI = float(np.pi)
TWO_PI = float(2 * np.pi)
GROUPS = [[0, 1, 2, 3], [4, 5, 6, 7]]      # batch groups (seq AG, out RS)
WGROUPS = [[0, 4], [1, 5], [2, 6], [3, 7]]  # batch pairs (weight AG)


def _split_sync_waits(nc, limit=1):
    """This container's walrus rejects >1 sync-wait per instruction; move
    excess waits onto same-engine NOPs inserted just before."""
    ctr = 0
    for f in nc.m.functions:
        for bb in f.blocks:
            il = bb.instructions
            i = 0
            while i < len(il):
                inst = il[i]
                si = getattr(inst, "sync_info", None)
                if si is None:
                    i += 1
                    continue
                waits = list(si.on_wait)
                if len(waits) <= limit:
                    i += 1
                    continue
                keep, rest = waits[:limit], waits[limit:]
                nops = []
                for j in range(0, len(rest), limit):
                    ctr += 1
                    nop = mybir.InstNoOp(name=f"I-wsplit-{ctr}", ins=[], outs=[])
                    nop.engine = inst.engine
                    nop.sync_info = mybir.SyncInfo(
                        on_update=[], on_wait=rest[j:j + limit])
                    nops.append(nop)
                si.on_wait = keep
                inst.sync_info = si
                for k, nop in enumerate(nops):
                    il.insert(i + k, nop)
                i += len(nops) + 1
            bb.instructions = il


def emit_mha(nc, tc, s_len=S, chunk=512, kb=3, reps=1):
    """Emit the per-core MHA kernel body. s_len tokens, q-chunks of
    `chunk`, exp batches of `kb` k-tiles. reps>1 re-emits the body for
    wall-clock delta timing."""
    T = s_len // 128           # s-tiles
    MT = M // 128              # m-tiles of the model dim
    NJ = s_len // chunk        # q chunks
    HD = HL * D                # 256
    sq = s_len // 4            # per-core sequence quarter

    xq = nc.declare_dram_parameter("xq", [M, sq], BF16, isOutput=False)
    # weight slices are identical across the two batches: each core sends
    # its batch's flat half, AllGather over batch pairs rebuilds the full
    # slice on device
    wqkvh = nc.declare_dram_parameter(
        "wqkvh", [M // 2, HD + 2 * D], BF16, isOutput=False)
    woh = nc.declare_dram_parameter("woh", [HD // 2, M], BF16, isOutput=False)
    qpos = nc.declare_dram_parameter("qpos", [128, 2 * T], F32, isOutput=False)
    kpos = nc.declare_dram_parameter("kpos", [128, 2 * T], F32, isOutput=False)
    invf = nc.declare_dram_parameter("invf", [128, 16], F32, isOutput=False)
    iden = nc.declare_dram_parameter("iden", [128, 128], BF16, isOutput=False)
    out = nc.declare_dram_parameter("out", [sq, M], BF16, isOutput=True)

    for _ in range(reps):
        _emit_body(nc, tc, s_len, chunk, kb, T, MT, NJ, HD, sq,
                   xq, wqkvh, woh, qpos, kpos, invf, iden, out)


def _emit_body(nc, tc, s_len, chunk, kb, T, MT, NJ, HD, sq,
               xq, wqkvh, woh, qpos, kpos, invf, iden, out):
    with tc.tile_pool(name="persist", bufs=1) as pp, \
         tc.tile_pool(name="dram", bufs=1, space="DRAM") as dp:
        # ---- DRAM bounce buffers for collectives ----
        xin_b = dp.tile([M, sq], BF16, tag="xin_b")
        xg_b = dp.tile([4, M, sq], BF16, tag="xg_b")
        wqh_b = dp.tile([M // 2, HD + 2 * D], BF16, tag="wqh_b")
        wqg_b = dp.tile([M, HD + 2 * D], BF16, tag="wqg_b")
        woh_b = dp.tile([HD // 2, M], BF16, tag="woh_b")
        wog_b = dp.tile([HD, M], BF16, tag="wog_b")
        po_b = dp.tile([s_len, M], F32, tag="po_b")
        rs_b = dp.tile([sq, M], F32, tag="rs_b")

        # ---- persistent SBUF ----
        xqt_sb = pp.tile([128, MT, s_len], BF16, tag="xqt")
        wqkv_sb = pp.tile([128, MT, HD + 2 * D], BF16, tag="wqkv")
        wo_sb = pp.tile([128, HD // 128, M], BF16, tag="wo")
        qpos_sb = pp.tile([128, T, 2], F32, tag="qpos")
        kpos_sb = pp.tile([128, T, 2], F32, tag="kpos")
        invf_sb = pp.tile([128, 16], F32, tag="invf")
        iden_sb = pp.tile([128, 128], BF16, tag="iden")

        nc.sync.dma_start(qpos_sb[:], qpos.rearrange("p (t c) -> p t c", c=2))
        nc.sync.dma_start(kpos_sb[:], kpos.rearrange("p (t c) -> p t c", c=2))
        nc.sync.dma_start(invf_sb[:], invf[:])
        nc.sync.dma_start(iden_sb[:], iden[:])

        # AllGather the 4 sequence-quarters of x[b]^T within the batch group.
        # Flat semantics: xg_b[r] = rank r's [M, sq] quarter = columns
        # r*sq..(r+1)*sq of the full [M, S] xqt.
        nc.sync.dma_start(xin_b[:], xq[:])
        nc.gpsimd.collective_compute(
            "AllGather", ALU.bypass, replica_groups=GROUPS,
            ins=[xin_b[:]], outs=[xg_b[:]])
        # AllGather the flat row-halves of the weight slices across the
        # batch pair (rank 0 = batch 0's core, rank 1 = batch 1's).
        nc.sync.dma_start(wqh_b[:], wqkvh[:])
        nc.gpsimd.collective_compute(
            "AllGather", ALU.bypass, replica_groups=WGROUPS,
            ins=[wqh_b[:]], outs=[wqg_b[:]])
        nc.sync.dma_start(woh_b[:], woh[:])
        nc.gpsimd.collective_compute(
            "AllGather", ALU.bypass, replica_groups=WGROUPS,
            ins=[woh_b[:]], outs=[wog_b[:]])
        nc.sync.dma_start(
            wqkv_sb[:], wqg_b[:].rearrange("(mt p) n -> p mt n", p=128))
        nc.sync.dma_start(
            wo_sb[:], wog_b[:].rearrange("(k p) n -> p k n", p=128))
        for r in range(4):
            nc.sync.dma_start(
                xqt_sb[:, :, r * sq:(r + 1) * sq],
                xg_b[r:r + 1].squeeze(0).rearrange("(mt p) s -> p mt s", p=128))

        # constants
        ones64 = pp.tile([128, 64], BF16, tag="ones64")
        nc.vector.memset(ones64[:], 1.0)

        # ---- rope tables: cos/sin for q and k, [128, T, 2, 16] bf16 ----
        tabs = {}
        with tc.tile_pool(name="tabtmp", bufs=2) as tp:
            for nm, pos_sb in (("q", qpos_sb), ("k", kpos_sb)):
                freq = tp.tile([128, T * 32], F32, tag="freq")
                nc.vector.tensor_tensor(
                    freq[:].rearrange("p (t c f) -> p t c f", c=2, f=16),
                    pos_sb[:].unsqueeze(3).broadcast_to((128, T, 2, 16)),
                    invf_sb[:].unsqueeze(1).unsqueeze(1)
                    .broadcast_to((128, T, 2, 16)),
                    ALU.mult)
                sarg = tp.tile([128, T * 32], F32, tag="sarg")
                carg = tp.tile([128, T * 32], F32, tag="carg")
                ge = tp.tile([128, T * 32], F32, tag="ge")
                yi = tp.tile([128, T * 32], mybir.dt.int32, tag="yi")
                yf = tp.tile([128, T * 32], F32, tag="yf")
                # m = freq - 2pi*int(freq/2pi)  (freq >= 0)
                nc.vector.tensor_scalar(yf[:], freq[:], 1.0 / TWO_PI, None,
                                        op0=ALU.mult)
                nc.vector.tensor_copy(yi[:], yf[:])
                nc.vector.tensor_copy(yf[:], yi[:])
                m = freq
                nc.vector.scalar_tensor_tensor(m[:], yf[:], -TWO_PI, freq[:],
                                               op0=ALU.mult, op1=ALU.add)
                # sarg = wrap(m) into [-pi, pi]
                nc.vector.tensor_scalar(ge[:], m[:], PI, None, op0=ALU.is_gt)
                nc.vector.scalar_tensor_tensor(sarg[:], ge[:], -TWO_PI, m[:],
                                               op0=ALU.mult, op1=ALU.add)
                # carg = wrap(m + pi/2)
                nc.vector.tensor_scalar(carg[:], m[:], PI / 2, None, op0=ALU.add)
                nc.vector.tensor_scalar(ge[:], carg[:], PI, None, op0=ALU.is_gt)
                nc.vector.scalar_tensor_tensor(carg[:], ge[:], -TWO_PI, carg[:],
                                               op0=ALU.mult, op1=ALU.add)
                sin_t = pp.tile([128, T * 32], BF16, tag=f"sin_{nm}")
                cos_t = pp.tile([128, T * 32], BF16, tag=f"cos_{nm}")
                nc.scalar.activation(sin_t[:], sarg[:], ACT.Sin)
                nc.scalar.activation(cos_t[:], carg[:], ACT.Sin)
                tabs[nm] = (cos_t, sin_t)

        # ---- projection + ssq ----
        qkv_sb = [pp.tile([128, 6, 64], F32, tag=f"qkv{t}", name=f"qkv{t}")
                  for t in range(T)]
        allssq = pp.tile([128, T, 6], F32, tag="allssq")
        invrms = pp.tile([128, T, 6], F32, tag="invrms")
        epsb = pp.tile([128, 1], F32, tag="epsb")
        nc.vector.memset(epsb[:], 1e-6)
        with tc.tile_pool(name="psum_proj", bufs=2, space="PSUM") as prp, \
             tc.tile_pool(name="sqtmp", bufs=2) as sqp:
            for t in range(T):
                ps = prp.tile([128, HD + 2 * D], F32, tag="proj")
                for m in range(MT):
                    nc.tensor.matmul(
                        ps[:], xqt_sb[:, m, t * 128:(t + 1) * 128],
                        wqkv_sb[:, m, :],
                        start=(m == 0), stop=(m == MT - 1))
                nc.any.tensor_copy(
                    qkv_sb[t][:], ps[:].rearrange("p (h d) -> p h d", d=64))
                sq_t = sqp.tile([128, 6, 64], F32, tag="sq")
                nc.vector.tensor_tensor(sq_t[:], qkv_sb[t][:], qkv_sb[t][:],
                                        ALU.mult)
                nc.vector.tensor_reduce(
                    allssq[:, t:t + 1, :].rearrange("p a b -> p (a b)"),
                    sq_t[:], axis=mybir.AxisListType.X, op=ALU.add)
                # invrms = rsqrt(ssq/64 + eps) per half, to unblock rope early
                if t == T // 2 - 1 or t == T - 1:
                    lo = 0 if t < T // 2 else T // 2
                    sl = (slice(None), slice(lo, t + 1), slice(None))
                    nc.scalar.activation(invrms[sl], allssq[sl], ACT.Ln,
                                         scale=1.0 / 64.0, bias=epsb[:])
                    nc.scalar.activation(invrms[sl], invrms[sl], ACT.Exp,
                                         scale=-0.5)
                    nc.vector.memset(invrms[:, lo:t + 1, 5:6], 1.0)

        # ---- norm + rope + transpose ----
        qt_sb = [pp.tile([128, s_len], BF16, tag=f"qt{h}", name=f"qt{h}")
                 for h in range(HL)]
        kt_sb = pp.tile([128, s_len], BF16, tag="kt")
        vb = [pp.tile([128, 64], BF16, tag=f"v{t}", name=f"v{t}") for t in range(T)]
        (cq, sq_tab), (ck, sk) = tabs["q"], tabs["k"]
        with tc.tile_pool(name="rope", bufs=3) as rp, \
             tc.tile_pool(name="psum_tr", bufs=4, space="PSUM") as trp:
            for t in range(T):
                qkvbf = rp.tile([128, 6, 64], BF16, tag="qkvbf")
                nc.vector.tensor_tensor(
                    qkvbf[:], qkv_sb[t][:],
                    invrms[:, t:t + 1, :].rearrange("p a b -> p (a b)")
                    .unsqueeze(2).broadcast_to((128, 6, 64)),
                    ALU.mult)
                nc.any.tensor_copy(vb[t][:], qkvbf[:, 5:6, :].squeeze(1))
                qro = rp.tile([128, 5, 64], BF16, tag="qro")
                tmp1 = rp.tile([128, 128], BF16, tag="tmp1")
                tmp2 = rp.tile([128, 128], BF16, tag="tmp2")
                for nm, h0, nh, (cos_t, sin_t) in (
                        ("q", 0, HL, (cq, sq_tab)), ("k", HL, 1, (ck, sk))):
                    fl = qkvbf[:, h0:h0 + nh, :].rearrange(
                        "p h (c u f) -> p h c u f", c=2, u=2)
                    a1 = fl[:, :, :, 0:1, :].squeeze(3)
                    a2 = fl[:, :, :, 1:2, :].squeeze(3)
                    ro = qro[:, h0:h0 + nh, :].rearrange(
                        "p h (c u f) -> p h c u f", c=2, u=2)
                    o1 = ro[:, :, :, 0:1, :].squeeze(3)
                    o2 = ro[:, :, :, 1:2, :].squeeze(3)
                    cosv = cos_t[:, t * 32:(t + 1) * 32] \
                        .rearrange("p (c f) -> p c f", f=16).unsqueeze(1) \
                        .broadcast_to((128, nh, 2, 16))
                    sinv = sin_t[:, t * 32:(t + 1) * 32] \
                        .rearrange("p (c f) -> p c f", f=16).unsqueeze(1) \
                        .broadcast_to((128, nh, 2, 16))
                    w1 = tmp1[:, 0:nh * 32].rearrange(
                        "p (h c f) -> p h c f", c=2, f=16)
                    w2 = tmp2[:, 0:nh * 32].rearrange(
                        "p (h c f) -> p h c f", c=2, f=16)
                    nc.vector.tensor_tensor(w1, a1, cosv, ALU.mult)
                    nc.vector.tensor_tensor(w2, a2, sinv, ALU.mult)
                    nc.vector.tensor_tensor(o1, w1, w2, ALU.subtract)
                    nc.vector.tensor_tensor(w1, a2, cosv, ALU.mult)
                    nc.vector.tensor_tensor(w2, a1, sinv, ALU.mult)
                    nc.vector.tensor_tensor(o2, w1, w2, ALU.add)
                for h in range(HL + 1):
                    dst = kt_sb if h == HL else qt_sb[h]
                    pt = trp.tile([64, 128], BF16, tag="tr")
                    nc.tensor.transpose(
                        pt[:], qro[:, h:h + 1, :].squeeze(1), iden_sb[:])
                    nc.any.tensor_copy(
                        dst[0:64, t * 128:(t + 1) * 128], pt[:])
        # duplicate to partitions 64:128 for row-group packing
        for h in range(HL):
            nc.vector.tensor_copy(qt_sb[h][64:128, :], qt_sb[h][0:64, :])
        nc.vector.tensor_copy(kt_sb[64:128, :], kt_sb[0:64, :])

        # ---- attention ----
        out_t = [pp.tile([128, s_len], BF16, tag=f"outT{hp}", name=f"outT{hp}")
                 for hp in range(HL // 2)]
        kts = list(range(T))
        batches = [kts[i:i + kb] for i in range(0, T, kb)]
        with tc.tile_pool(name="sc", bufs=2, space="PSUM") as scp, \
             tc.tile_pool(name="av", bufs=1, space="PSUM") as avp, \
             tc.tile_pool(name="se", bufs=1, space="PSUM") as sep, \
             tc.tile_pool(name="expt", bufs=4) as ep, \
             tc.tile_pool(name="smtmp", bufs=2) as smp:
            for j in range(NJ):
                for hp in range(HL // 2):
                    se = sep.tile([128, chunk], F32, tag="se")
                    avt = avp.tile([128, chunk], F32, tag="av")
                    expts = {}
                    for bi, batch in enumerate(batches):
                        for hh in range(2):
                            h = 2 * hp + hh
                            sc = scp.tile([128, kb * chunk], F32, tag="sc")
                            for ki, kt in enumerate(batch):
                                rg = kt % 2
                                nc.tensor.matmul(
                                    sc[:, ki * chunk:(ki + 1) * chunk],
                                    kt_sb[rg * 64:(rg + 1) * 64,
                                          kt * 128:(kt + 1) * 128],
                                    qt_sb[h][rg * 64:(rg + 1) * 64,
                                             j * chunk:(j + 1) * chunk],
                                    start=True, stop=True,
                                    tile_position=(rg * 64, 0))
                            et = ep.tile([128, kb * chunk], BF16, tag="expt")
                            nc.scalar.activation(
                                et[:, 0:len(batch) * chunk],
                                sc[:, 0:len(batch) * chunk],
                                ACT.Exp, scale=0.125)
                            expts[hh] = et
                        for ki, kt in enumerate(batch):
                            for hh in range(2):
                                h = 2 * hp + hh
                                nc.tensor.matmul(
                                    avt[hh * 64:(hh + 1) * 64, :],
                                    vb[kt][:],
                                    expts[hh][:, ki * chunk:(ki + 1) * chunk],
                                    start=(kt == 0), stop=(kt == T - 1),
                                    tile_position=(0, hh * 64),
                                    skip_group_check=True)
                                nc.tensor.matmul(
                                    se[hh * 64:(hh + 1) * 64, :],
                                    ones64[:],
                                    expts[hh][:, ki * chunk:(ki + 1) * chunk],
                                    start=(kt == 0), stop=(kt == T - 1),
                                    tile_position=(0, hh * 64),
                                    skip_group_check=True)
                    # 1/sumexp via exp(-ln(x)); se rows already replicated
                    # across each head's 64 partitions
                    rec = smp.tile([128, chunk], F32, tag="rec")
                    nc.scalar.activation(rec[:], se[:], ACT.Ln)
                    nc.scalar.activation(rec[:], rec[:], ACT.Exp, scale=-1.0)
                    nc.vector.tensor_tensor(
                        out_t[hp][:, j * chunk:(j + 1) * chunk],
                        avt[:], rec[:], ALU.mult)

        # ---- O-projection (partial through the Wo row-slice) ----
        with tc.tile_pool(name="psum_o", bufs=4, space="PSUM") as pop, \
             tc.tile_pool(name="ostage", bufs=3) as osp:
            for t in range(T):
                ost = osp.tile([128, M], F32, tag="ost")
                for n in range(M // 512):
                    po = pop.tile([128, 512], F32, tag="po")
                    for k in range(HD // 128):
                        nc.tensor.matmul(
                            po[:], out_t[k][:, t * 128:(t + 1) * 128],
                            wo_sb[:, k, n * 512:(n + 1) * 512],
                            start=(k == 0), stop=(k == HD // 128 - 1))
                    nc.any.tensor_copy(ost[:, n * 512:(n + 1) * 512], po[:])
                nc.sync.dma_start(po_b[t * 128:(t + 1) * 128, :], ost[:])

        # ---- on-device sum of the 4 head-group partials; rank g keeps
        # final output rows g*sq..(g+1)*sq of its batch ----
        nc.gpsimd.collective_compute(
            "ReduceScatter", ALU.add, replica_groups=GROUPS,
            ins=[po_b[:]], outs=[rs_b[:]])
        with tc.tile_pool(name="oconv", bufs=2) as ocp:
            obf = ocp.tile([128, 4, M], BF16, tag="obf")
            rs_r = rs_b.rearrange("(t p) n -> p t n", p=128)
            for t in range(4):
                of32 = ocp.tile([128, M], F32, tag="of32")
                nc.sync.dma_start(of32[:], rs_r[:, t:t + 1, :].squeeze(1))
                nc.vector.tensor_copy(obf[:, t, :], of32[:])
            nc.sync.dma_start(out.rearrange("(t p) n -> p t n", p=128), obf[:])


_NC_CACHE = {}


def _build(s_len=S, chunk=512, kb=3, reps=1):
    key = (s_len, chunk, kb, reps)
    if key not in _NC_CACHE:
        nc = bass.Bass()
        with tile.TileContext(nc) as tc:
            emit_mha(nc, tc, s_len=s_len, chunk=chunk, kb=kb, reps=reps)
        _split_sync_waits(nc)
        _NC_CACHE[key] = nc
    return _NC_CACHE[key]


def _prep_core_inputs(x_q, q_pos, k_pos, Wq, Wk, Wv, Wo, b, g, s_len=S):
    T = s_len // 128
    sq = s_len // 4
    bf = ml_dtypes.bfloat16
    xq = np.ascontiguousarray(x_q[b, g * sq:(g + 1) * sq, :].T).astype(bf)
    m0, m1 = b * (M // 2), (b + 1) * (M // 2)
    wqkvh = np.concatenate(
        [Wq[m0:m1, 4 * g:4 * g + 4, :].reshape(M // 2, HL * D),
         Wk[m0:m1, g, :], Wv[m0:m1, g, :]], axis=1).astype(bf)
    k0 = HL * D * g
    woh = Wo[k0 + b * (HL * D // 2):k0 + (b + 1) * (HL * D // 2), :].astype(bf)
    qp = q_pos[b].astype(np.float32).reshape(T, 128, 2) \
        .transpose(1, 0, 2).reshape(128, 2 * T)
    kp = k_pos[b].astype(np.float32).reshape(T, 128, 2) \
        .transpose(1, 0, 2).reshape(128, 2 * T)
    invf = (10000.0 ** (-np.arange(0, 32, 2, dtype=np.float32) / 32.0))
    invf = np.broadcast_to(invf[None, :], (128, 16)).copy()
    iden = np.eye(128, dtype=bf)
    return {"xq": xq,
            "wqkvh": np.ascontiguousarray(wqkvh),
            "woh": np.ascontiguousarray(woh),
            "qpos": np.ascontiguousarray(qp),
            "kpos": np.ascontiguousarray(kp),
            "invf": invf, "iden": iden}


def kernel(x_q, q_pos, k_pos, Wq, Wk, Wv, Wo):
    x_q, q_pos, k_pos = np.asarray(x_q), np.asarray(q_pos), np.asarray(k_pos)
    Wq, Wk, Wv, Wo = (np.asarray(w) for w in (Wq, Wk, Wv, Wo))
    nc = _build()
    in_maps = [
        _prep_core_inputs(x_q, q_pos, k_pos, Wq, Wk, Wv, Wo, c // 4, c % 4)
        for c in range(8)]
    res = bass_utils.run_bass_kernel_spmd(nc, in_maps, core_ids=list(range(8)))
    out = np.empty((B, S, M), np.float32)
    for c in range(8):
        b, g = c // 4, c % 4
        out[b, g * SQ:(g + 1) * SQ, :] = np.asarray(
            res.results[c]["out"], dtype=np.float32)
    return out
